# revision 1
# baseline (speedup 1.0000x reference)
"""Tensor-parallel Llama attention for 8 TRN2 NeuronCores.

Sharding: core d handles batch d//4 and q-head group g = d%4 (q heads
4g..4g+3, kv head g — GQA group-aligned so each core needs exactly one
kv head).  Wq/Wk/Wv are row-sharded, Wo column-sharded; the per-batch
partial o_proj outputs of 4 cores are summed on the host.

Device layouts (prepared host-side, bf16):
  hsT  [16,128,S]   hidden_states[b].T, HID on partitions in 16 chunks
  wqT  [16,128,512] Wq_shard.T          wkT/wvT [16,128,128]
  woT  [4,128,2048] Wo_shard.T (4 contraction chunks of the 512 local dims)
  cosT/sinT [128,S] RoPE tables in [head_dim, seq] layout
  mask [4,128,512]  0/1 causal masks for the 4 diagonal-block phases

Compute: q/k kept transposed [d, s] for scores; v transposed back to
[s, d] via PE transpose for PV; scores computed as scoresT [j, i] so
softmax probs feed PV directly without transposition.  Softmax sums via
ones-vector matmul over partitions (no max subtraction: inputs are
N(0,~0.8) scores, exp stays far below f32 overflow).
"""

import sys

sys.path.insert(0, "/opt/trn_rl_repo")

import numpy as np
import ml_dtypes

B, S, HID = 2, 2048, 2048
NH, NKV, HD = 16, 4, 128
THETA = 10000.0
NCORES = 8
HPC = 4            # q heads per core
QDIM = HPC * HD    # 512 local q dims
KT = HID // 128    # 16 contraction chunks
SB = S // 512      # 4 column groups of 512
ST = S // 128      # 16 row tiles of 128

_CACHE = {}


def _patch_tile_drain():
    """This walrus build caps sync waits per CTRL instruction below what the
    stock Tile kernel-tail drain carries; split them into single-wait NOPs."""
    import bass_rust
    import concourse.tile as tile
    from concourse.tile import ScopedClock

    if getattr(tile.TileContext, "_drain_split_patched", False):
        return

    def _split_drain_and_barrier(self, tick_clock, wait_clock):
        ticks = list(tick_clock.global_clock)
        for i, v in enumerate(ticks):
            if v > 0:
                single = [0] * len(ticks)
                single[i] = v
                nop = self.nc.sync.nop(nofuse=True, hint=f"drain_wait_{i}")
                wait_clock.add_sem_waits(
                    nop.ins, ScopedClock({None: bass_rust.VectorClock(single)})
                )
        self.nc.sync.drain()
        self.nc.all_engine_barrier()
        assert self.sems is not None
        popped = self.nc._tile_sem_poison_stack.pop()
        assert popped is self._sem_poison
        self.nc.clear_and_free_semaphores(list(self.sems.allocated().values()))
        self.nc.all_engine_barrier()

    tile.TileContext._drain_and_barrier = _split_drain_and_barrier
    tile.TileContext._drain_split_patched = True


def _legalize_waits(nc, max_waits=1):
    """This walrus build rejects instructions carrying more than ~2 sync
    waits.  Hoist the excess onto single-wait NOPs inserted just before the
    instruction in its block (same engine => same instruction stream, so
    the waits still complete before the op issues)."""
    import concourse.mybir as mybir

    n_split = 0
    for block in nc.m.functions[0].blocks:
        insts = list(block.instructions)
        out = []
        for inst in insts:
            si = getattr(inst, "sync_info", None)
            if si is not None and si.on_wait and len(si.on_wait) > max_waits:
                waits = list(si.on_wait)
                keep = waits[:max_waits]
                for j, w in enumerate(waits[max_waits:]):
                    out.append(
                        mybir.InstNoOp(
                            name=f"{inst.name}_hw{j}",
                            engine=inst.engine,
                            bass_nofuse=True,
                            sync_info=mybir.SyncInfo(on_wait=[w], on_update=[]),
                        )
                    )
                si.on_wait = keep
                n_split += 1
            out.append(inst)
        block.instructions = out
    return n_split


def _build_nc():
    import concourse.bass as bass
    import concourse.mybir as mybir
    import concourse.tile as tile
    from concourse.masks import make_identity

    _patch_tile_drain()

    bf = mybir.dt.bfloat16
    f32 = mybir.dt.float32
    Exp = mybir.ActivationFunctionType.Exp

    nc = bass.Bass()
    hsT = nc.declare_dram_parameter("hsT", [KT, 128, S], bf, isOutput=False)
    wqT = nc.declare_dram_parameter("wqT", [KT, 128, QDIM], bf, isOutput=False)
    wkT = nc.declare_dram_parameter("wkT", [KT, 128, HD], bf, isOutput=False)
    wvT = nc.declare_dram_parameter("wvT", [KT, 128, HD], bf, isOutput=False)
    woT = nc.declare_dram_parameter("woT", [4, 128, HID], bf, isOutput=False)
    cosT = nc.declare_dram_parameter("cosT", [128, S], bf, isOutput=False)
    sinT = nc.declare_dram_parameter("sinT", [128, S], bf, isOutput=False)
    mask = nc.declare_dram_parameter("mask", [4, 128, 512], bf, isOutput=False)
    out = nc.declare_dram_parameter("out", [S, HID], f32, isOutput=True)

    with tile.TileContext(nc) as tc:
        with (
            tc.tile_pool(name="resid", bufs=1) as resid,
            tc.tile_pool(name="probs", bufs=6) as probs_pool,
            tc.tile_pool(name="rc", bufs=2) as rc_pool,
            tc.tile_pool(name="bc", bufs=2) as bc_pool,
            tc.tile_pool(name="ostage", bufs=4) as ostage_pool,
            tc.tile_pool(name="mm_ps", bufs=2, space="PSUM") as mm_ps,
            tc.tile_pool(name="score_ps", bufs=2, space="PSUM") as score_ps,
            tc.tile_pool(name="pv_ps", bufs=2, space="PSUM") as pv_ps,
            tc.tile_pool(name="sum_ps", bufs=2, space="PSUM") as sum_ps,
        ):
            hs_sb = resid.tile([128, KT * S], bf)
            wq_sb = resid.tile([128, KT * QDIM], bf)
            wk_sb = resid.tile([128, KT * HD], bf)
            wv_sb = resid.tile([128, KT * HD], bf)
            wo_sb = resid.tile([128, 4 * HID], bf)
            cos_sb = resid.tile([128, S], bf)
            sin_sb = resid.tile([128, S], bf)
            mask_sb = resid.tile([128, 4 * 512], bf)
            ones_sb = resid.tile([128, 1], bf)
            ones4_sb = resid.tile([4, 128], f32)
            ident = resid.tile([128, 128], bf)
            qT_sb = resid.tile([128, HPC * S], bf)
            kT_sb = resid.tile([128, S], bf)
            vT_sb = resid.tile([128, S], bf)
            vn_sb = resid.tile([128, S], bf)
            at_sb = resid.tile([128, HPC * S], bf)
            rot_sb = resid.tile([128, S], bf)

            # ---- load everything ----
            for kk in range(KT):
                nc.sync.dma_start(hs_sb[:, kk * S:(kk + 1) * S], hsT[kk])
                nc.sync.dma_start(wq_sb[:, kk * QDIM:(kk + 1) * QDIM], wqT[kk])
                nc.sync.dma_start(wk_sb[:, kk * HD:(kk + 1) * HD], wkT[kk])
                nc.sync.dma_start(wv_sb[:, kk * HD:(kk + 1) * HD], wvT[kk])
            for c in range(4):
                nc.sync.dma_start(wo_sb[:, c * HID:(c + 1) * HID], woT[c])
                nc.sync.dma_start(mask_sb[:, c * 512:(c + 1) * 512], mask[c])
            nc.sync.dma_start(cos_sb[:], cosT[:])
            nc.sync.dma_start(sin_sb[:], sinT[:])
            nc.gpsimd.memset(ones_sb[:], 1.0)
            nc.gpsimd.memset(ones4_sb[:], 1.0)
            make_identity(nc, ident[:])

            # ---- q/k/v projections (transposed layouts) ----
            def project(w_sb, wdim, mtiles, dst, dst_stride):
                # dst[:, m*dst_stride + sg*512 ...] = (W.T chunk m) over s
                for m in range(mtiles):
                    for sg in range(SB):
                        ps = mm_ps.tile([128, 512], f32, tag="mm")
                        for kk in range(KT):
                            nc.tensor.matmul(
                                ps[:],
                                w_sb[:, kk * wdim + m * 128: kk * wdim + (m + 1) * 128],
                                hs_sb[:, kk * S + sg * 512: kk * S + sg * 512 + 512],
                                start=(kk == 0),
                                stop=(kk == KT - 1),
                            )
                        nc.vector.tensor_copy(
                            dst[:, m * dst_stride + sg * 512: m * dst_stride + sg * 512 + 512],
                            ps[:],
                        )

            # k/v first so attention can start while q heads still project
            project(wk_sb, HD, 1, kT_sb, S)
            project(wv_sb, HD, 1, vT_sb, S)

            # ---- v back to natural [s, d] layout via PE transpose ----
            for tj in range(ST):
                tp = mm_ps.tile([128, 128], bf, tag="mm")
                nc.tensor.transpose(tp[:], vT_sb[:, tj * 128:(tj + 1) * 128], ident[:])
                nc.vector.tensor_copy(vn_sb[:, tj * 128:(tj + 1) * 128], tp[:])

            project(wq_sb, QDIM, HPC, qT_sb, S)

            # ---- RoPE (in place, [d, s] layout); k first ----
            def rope(h):
                nc.vector.tensor_scalar_mul(rot_sb[0:64, :], h[64:128, :], -1.0)
                nc.vector.tensor_copy(rot_sb[64:128, :], h[0:64, :])
                nc.vector.tensor_mul(h, h, cos_sb[:])
                nc.vector.tensor_mul(rot_sb[:], rot_sb[:], sin_sb[:])
                nc.vector.tensor_add(h, h, rot_sb[:])

            rope(kT_sb[:])
            for h in range(HPC):
                rope(qT_sb[:, h * S:(h + 1) * S])

            # ---- attention (gi-outer so o_proj interleaves per i-group) ----
            inv_sqrt_d = 1.0 / float(np.sqrt(HD))
            for gi in range(SB):
                ntj = 4 * gi + 4
                pack = rc_pool.tile([128, 512], f32, tag="pack")
                for h in range(HPC):
                    qh = qT_sb[:, h * S:(h + 1) * S]
                    pv = pv_ps.tile([128, 512], f32)
                    sm = sum_ps.tile([1, 512], f32)
                    for tj in range(ntj):
                        sc = score_ps.tile([128, 512], f32)
                        nc.tensor.matmul(
                            sc[:],
                            kT_sb[:, tj * 128:(tj + 1) * 128],
                            qh[:, gi * 512:gi * 512 + 512],
                            start=True,
                            stop=True,
                        )
                        pb = probs_pool.tile([128, 512], bf)
                        nc.scalar.activation(pb[:], sc[:], Exp, scale=inv_sqrt_d)
                        if tj >= 4 * gi:  # diagonal block: causal 0/1 mask
                            p = tj - 4 * gi
                            nc.vector.tensor_mul(
                                pb[:], pb[:], mask_sb[:, p * 512:(p + 1) * 512]
                            )
                        nc.tensor.matmul(
                            sm[:], ones_sb[:], pb[:],
                            start=(tj == 0), stop=(tj == ntj - 1),
                        )
                        nc.tensor.matmul(
                            pv[:], vn_sb[:, tj * 128:(tj + 1) * 128], pb[:],
                            start=(tj == 0), stop=(tj == ntj - 1),
                        )
                    # stage unnormalized pv in at_sb; pack the sums row
                    nc.vector.tensor_copy(
                        at_sb[:, h * S + gi * 512: h * S + gi * 512 + 512], pv[:]
                    )
                    nc.vector.tensor_copy(pack[32 * h:32 * h + 1, :], sm[:])
                # one batched reciprocal for the 4 heads of this i-group
                # (rows live at 32-aligned partitions; other rows are junk)
                rcp = rc_pool.tile([128, 512], f32, tag="rcp")
                nc.vector.reciprocal(rcp[:], pack[:])
                for h in range(HPC):
                    rc1 = rc_pool.tile([1, 512], f32, tag="rc1")
                    nc.vector.tensor_copy(rc1[:], rcp[32 * h:32 * h + 1, :])
                    bc_ps = mm_ps.tile([128, 512], f32, tag="mm")
                    nc.tensor.matmul(
                        bc_ps[:], ones4_sb[0:1, :], rc1[:],
                        start=True, stop=True,
                    )
                    bc = bc_pool.tile([128, 512], f32)
                    nc.vector.tensor_copy(bc[:], bc_ps[:])
                    a_sl = at_sb[:, h * S + gi * 512: h * S + gi * 512 + 512]
                    nc.vector.tensor_mul(a_sl, a_sl, bc[:])
                # o_proj for the 4 s-tiles covered by this i-group
                for st in range(4 * gi, 4 * gi + 4):
                    for eg in range(SB):
                        ps = mm_ps.tile([128, 512], f32, tag="mm")
                        for h in range(HPC):
                            nc.tensor.matmul(
                                ps[:],
                                at_sb[:, h * S + st * 128: h * S + st * 128 + 128],
                                wo_sb[:, h * HID + eg * 512: h * HID + eg * 512 + 512],
                                start=(h == 0),
                                stop=(h == HPC - 1),
                            )
                        ostage = ostage_pool.tile([128, 512], f32)
                        nc.vector.tensor_copy(ostage[:], ps[:])
                        nc.sync.dma_start(
                            out[st * 128:(st + 1) * 128, eg * 512:(eg + 1) * 512],
                            ostage[:],
                        )
    _legalize_waits(nc)
    return nc


def _host_prep(hidden_states, Wq, Wk, Wv, Wo, position_ids):
    bf = ml_dtypes.bfloat16
    inv_freq = 1.0 / (THETA ** (np.arange(0, HD, 2, dtype=np.float64) / HD))

    mask = np.zeros((4, 128, 512), dtype=bf)
    jl = np.arange(128)[:, None]
    il = np.arange(512)[None, :]
    for p in range(4):
        mask[p] = (128 * p + jl <= il).astype(bf)

    in_maps = []
    for d in range(NCORES):
        b, g = d // 4, d % 4
        hsT = np.ascontiguousarray(hidden_states[b].T).astype(bf).reshape(KT, 128, S)
        wqT = np.ascontiguousarray(Wq[g * QDIM:(g + 1) * QDIM].T).astype(bf).reshape(KT, 128, QDIM)
        wkT = np.ascontiguousarray(Wk[g * HD:(g + 1) * HD].T).astype(bf).reshape(KT, 128, HD)
        wvT = np.ascontiguousarray(Wv[g * HD:(g + 1) * HD].T).astype(bf).reshape(KT, 128, HD)
        woT = np.ascontiguousarray(Wo[:, g * QDIM:(g + 1) * QDIM].T).astype(bf).reshape(4, 128, HID)
        freqs = position_ids[b].astype(np.float64)[:, None] * inv_freq[None, :]  # [S, 64]
        emb = np.concatenate([freqs, freqs], axis=1)  # [S, 128]
        cosT = np.cos(emb).T.astype(bf)
        sinT = np.sin(emb).T.astype(bf)
        in_maps.append({
            "hsT": hsT, "wqT": wqT, "wkT": wkT, "wvT": wvT, "woT": woT,
            "cosT": np.ascontiguousarray(cosT),
            "sinT": np.ascontiguousarray(sinT),
            "mask": mask,
        })
    return in_maps


def kernel(hidden_states, Wq, Wk, Wv, Wo, position_ids, _trace=False, _tmpdir=None):
    from concourse.bass_utils import run_bass_kernel_spmd

    if "nc" not in _CACHE:
        _CACHE["nc"] = _build_nc()
    nc = _CACHE["nc"]

    in_maps = _host_prep(
        np.asarray(hidden_states), np.asarray(Wq), np.asarray(Wk),
        np.asarray(Wv), np.asarray(Wo), np.asarray(position_ids),
    )
    res = run_bass_kernel_spmd(
        nc, in_maps, core_ids=list(range(NCORES)), trace=_trace, tmpdir=_tmpdir
    )
    _CACHE["last_result"] = res

    out = np.zeros((B, S, NH * HD), dtype=np.float32)
    for d in range(NCORES):
        out[d // 4] += res.results[d]["out"]
    return out



# revision 14
# speedup vs baseline: 1.3700x; 1.3700x over previous
"""Tensor-parallel Llama attention for 8 TRN2 NeuronCores.

Sharding: core d handles batch d//4 and q-head group g = d%4 (q heads
4g..4g+3, kv head g — GQA group-aligned so each core needs exactly one
kv head).  Wq/Wk/Wv are row-sharded, Wo column-sharded; the per-batch
partial o_proj outputs of 4 cores are summed on the host.

Device layouts (prepared host-side, bf16):
  hsT  [16,128,S]   hidden_states[b].T, HID on partitions in 16 chunks
  wqT  [16,128,512] Wq_shard.T          wkT/wvT [16,128,128]
  woT  [4,128,2048] Wo_shard.T (4 contraction chunks of the 512 local dims)
  cosT/sinT [128,S] RoPE tables in [head_dim, seq] layout
  maskb [4,128,512] additive causal bias (0 / -1e9) for diagonal blocks

Performance structure (v2): the tensor engine p-state ramps to full
clock only under continuous execution, so the whole kernel is scheduled
as one dense PE instruction stream:
  - scores/PV stay in transposed layout; softmax sums accumulate on the
    vector engine (scalar_tensor_tensor, 2x SBUF mode) instead of
    per-block ones-matmuls; a single f32r fold matmul + f32r broadcast
    matmul per (gi, head) recovers the denominators.
  - causal masking is an additive bias preloaded into PSUM before the
    (trimmed-width) diagonal score matmuls (start=False accumulation).
  - Q-projection chains and o_proj tiles are interleaved one matmul at
    a time between attention blocks so the PE never idles while the
    activation engine computes exp.
  - o_proj staging copies run on the (otherwise idle) Pool engine;
    outputs are written bf16; input DMAs are split across the SP and
    Activation hardware DGE queues.
"""

import sys

sys.path.insert(0, "/opt/trn_rl_repo")

import numpy as np
import ml_dtypes

B, S, HID = 2, 2048, 2048
NH, NKV, HD = 16, 4, 128
THETA = 10000.0
NCORES = 8
HPC = 4            # q heads per core
QDIM = HPC * HD    # 512 local q dims
KT = HID // 128    # 16 contraction chunks
SB = S // 512      # 4 column groups of 512
ST = S // 128      # 16 row tiles of 128

_CACHE = {}


def _patch_tile_drain():
    """This walrus build caps sync waits per CTRL instruction below what the
    stock Tile kernel-tail drain carries; split them into single-wait NOPs."""
    import bass_rust
    import concourse.tile as tile
    from concourse.tile import ScopedClock

    if getattr(tile.TileContext, "_drain_split_patched", False):
        return

    def _split_drain_and_barrier(self, tick_clock, wait_clock):
        ticks = list(tick_clock.global_clock)
        for i, v in enumerate(ticks):
            if v > 0:
                single = [0] * len(ticks)
                single[i] = v
                nop = self.nc.sync.nop(nofuse=True, hint=f"drain_wait_{i}")
                wait_clock.add_sem_waits(
                    nop.ins, ScopedClock({None: bass_rust.VectorClock(single)})
                )
        self.nc.sync.drain()
        self.nc.all_engine_barrier()
        assert self.sems is not None
        popped = self.nc._tile_sem_poison_stack.pop()
        assert popped is self._sem_poison
        self.nc.clear_and_free_semaphores(list(self.sems.allocated().values()))
        self.nc.all_engine_barrier()

    tile.TileContext._drain_and_barrier = _split_drain_and_barrier
    tile.TileContext._drain_split_patched = True


def _legalize_waits(nc, max_waits=1):
    """This walrus build rejects instructions carrying more than ~2 sync
    waits.  Hoist the excess onto single-wait NOPs inserted just before the
    instruction in its block (same engine => same instruction stream, so
    the waits still complete before the op issues)."""
    import concourse.mybir as mybir

    n_split = 0
    for block in nc.m.functions[0].blocks:
        insts = list(block.instructions)
        out = []
        for inst in insts:
            si = getattr(inst, "sync_info", None)
            if si is not None and si.on_wait and len(si.on_wait) > max_waits:
                waits = list(si.on_wait)
                keep = waits[:max_waits]
                for j, w in enumerate(waits[max_waits:]):
                    out.append(
                        mybir.InstNoOp(
                            name=f"{inst.name}_hw{j}",
                            engine=inst.engine,
                            bass_nofuse=True,
                            sync_info=mybir.SyncInfo(on_wait=[w], on_update=[]),
                        )
                    )
                si.on_wait = keep
                n_split += 1
            out.append(inst)
        block.instructions = out
    return n_split


def _build_nc():
    import concourse.bass as bass
    import concourse.mybir as mybir
    import concourse.tile as tile
    from concourse.masks import make_identity

    _patch_tile_drain()

    bf = mybir.dt.bfloat16
    f32 = mybir.dt.float32
    f32r = mybir.dt.float32r
    Exp = mybir.ActivationFunctionType.Exp
    MUL = mybir.AluOpType.mult
    ADD = mybir.AluOpType.add

    nc = bass.Bass()
    hsT = nc.declare_dram_parameter("hsT", [KT, 128, S], bf, isOutput=False)
    wqT = nc.declare_dram_parameter("wqT", [KT, 128, QDIM], bf, isOutput=False)
    wkT = nc.declare_dram_parameter("wkT", [KT, 128, HD], bf, isOutput=False)
    wvT = nc.declare_dram_parameter("wvT", [KT, 128, HD], bf, isOutput=False)
    woT = nc.declare_dram_parameter("woT", [4, 128, HID], bf, isOutput=False)
    cosT = nc.declare_dram_parameter("cosT", [128, S], bf, isOutput=False)
    sinT = nc.declare_dram_parameter("sinT", [128, S], bf, isOutput=False)
    maskb = nc.declare_dram_parameter("maskb", [4, 128, 512], bf, isOutput=False)
    out = nc.declare_dram_parameter("out", [S, HID], bf, isOutput=True)

    inv_sqrt_d = 1.0 / float(np.sqrt(HD))

    with tile.TileContext(nc) as tc:
        with (
            tc.tile_pool(name="resid", bufs=1) as resid,
            tc.tile_pool(name="probs", bufs=6) as probs_pool,
            tc.tile_pool(name="accp", bufs=2) as acc_pool,
            tc.tile_pool(name="rcp", bufs=2) as rc_pool,
            tc.tile_pool(name="ropes", bufs=2) as rope_pool,
            tc.tile_pool(name="ostage", bufs=4) as ostage_pool,
            tc.tile_pool(name="ps", bufs=1, space="PSUM") as ps,
        ):
            # PSUM bank budget (8 banks total):
            #   score 2 + qfill 1 + pv 2 + oproj 2 + small 1  == 8
            def ps_tile(tag, bufs, shape=(128, 512), dtype=f32):
                t = ps.tile(list(shape), dtype, tag=tag, bufs=bufs,
                            name=f"ps_{tag}")
                return t

            hs_sb = resid.tile([128, KT * S], bf)
            wq_sb = resid.tile([128, KT * QDIM], bf)
            wk_sb = resid.tile([128, KT * HD], bf)
            wv_sb = resid.tile([128, KT * HD], bf)
            wo_sb = resid.tile([128, 4 * HID], bf)
            cos_sb = resid.tile([128, S], bf)
            sin_sb = resid.tile([128, S], bf)
            maskb_sb = resid.tile([128, 4 * 512], bf)
            ones_tmp = resid.tile([128, 128], f32)
            ones_f32 = resid.tile([128, 1], f32r)
            onesr_sb = resid.tile([1, 128], f32r)
            ident = resid.tile([128, 128], bf)
            qT_sb = resid.tile([128, HPC * S], bf)
            kT_sb = resid.tile([128, S], bf)
            vT_sb = resid.tile([128, S], bf)
            vn_sb = resid.tile([128, S], bf)
            at_sb = resid.tile([128, HPC * S], bf)

            # ---- input DMAs, split across the two HW DGE queues ----
            # SP queue: even hs chunks, wk, wv, wo, maskb[0].
            # ACT queue: odd hs chunks, cos/sin, wq, maskb[1:].
            for kk in range(KT):
                nc.sync.dma_start(wk_sb[:, kk * HD:(kk + 1) * HD], wkT[kk])
            nc.sync.dma_start(maskb_sb[:, 0:512], maskb[0])
            nc.scalar.dma_start(cos_sb[:], cosT[:])
            nc.scalar.dma_start(sin_sb[:], sinT[:])
            for kk in range(KT):
                eng = nc.sync if kk % 2 == 0 else nc.scalar
                eng.dma_start(hs_sb[:, kk * S:(kk + 1) * S], hsT[kk])
            for kk in range(KT):
                nc.sync.dma_start(wv_sb[:, kk * HD:(kk + 1) * HD], wvT[kk])
            for kk in range(KT):
                nc.scalar.dma_start(
                    wq_sb[:, kk * QDIM:(kk + 1) * QDIM], wqT[kk])
            for c in range(4):
                nc.sync.dma_start(wo_sb[:, c * HID:(c + 1) * HID], woT[c])
            for p in range(1, 4):
                nc.scalar.dma_start(maskb_sb[:, p * 512:(p + 1) * 512],
                                    maskb[p])

            nc.gpsimd.memset(ones_tmp[:], 1.0)
            nc.vector.tensor_copy(ones_f32[:], ones_tmp[:, 0:1])
            nc.vector.tensor_copy(onesr_sb[:], ones_tmp[0:1, :])
            make_identity(nc, ident[:])

            # ---- RoPE on a [d, s-window] slice (in place, DVE) ----
            # dst_lo: column offset in the destination tile; s_lo: the
            # s-window it corresponds to (for the cos/sin tables).
            def rope(h, dst_lo, s_lo, width=512):
                dl = slice(dst_lo, dst_lo + width)
                sl = slice(s_lo, s_lo + width)
                rot = rope_pool.tile([128, 512], bf, tag="rot", name="rot")
                nc.vector.tensor_scalar_mul(
                    rot[0:64, 0:width], h[64:128, dl], -1.0)
                nc.vector.tensor_copy(rot[64:128, 0:width], h[0:64, dl])
                nc.vector.tensor_mul(h[:, dl], h[:, dl], cos_sb[:, sl])
                nc.vector.tensor_mul(
                    rot[:, 0:width], rot[:, 0:width], sin_sb[:, sl])
                nc.vector.tensor_add(h[:, dl], h[:, dl], rot[:, 0:width])

            # ---- K projection, kk-outer so it tracks DMA chunk arrival ----
            ktiles = [ps_tile("score", 2), ps_tile("score", 2),
                      ps_tile("qfill", 1), ps_tile("pv", 2)]
            for kk in range(KT):
                for sg in range(4):
                    nc.tensor.matmul(
                        ktiles[sg][:],
                        wk_sb[:, kk * HD:(kk + 1) * HD],
                        hs_sb[:, kk * S + sg * 512: kk * S + sg * 512 + 512],
                        start=(kk == 0), stop=(kk == KT - 1),
                    )
            for sg in range(4):
                nc.vector.tensor_copy(
                    kT_sb[:, sg * 512:(sg + 1) * 512], ktiles[sg][:])
            rope(kT_sb, 0, 0)

            # ---- generic 16-matmul projection chain -> dst slice ----
            # Yields micro-ops so chains can be interleaved as PE fillers.
            # m: 128-row block of W; s_lo: s-window; dst_lo: column offset
            # in dst where the [128, 512] result lands.
            def proj_chain_ops(w_sb, wdim, m, dst, dst_lo, s_lo, tag,
                               rope_after):
                def ops():
                    t = ps_tile(tag, 1 if tag == "qfill" else 2)
                    for kk in range(KT):
                        yield lambda kk=kk, t=t: nc.tensor.matmul(
                            t[:],
                            w_sb[:, kk * wdim + m * 128: kk * wdim + (m + 1) * 128],
                            hs_sb[:, kk * S + s_lo: kk * S + s_lo + 512],
                            start=(kk == 0), stop=(kk == KT - 1),
                        )

                    def fin(t=t):
                        nc.vector.tensor_copy(dst[:, dst_lo:dst_lo + 512],
                                              t[:])
                        if rope_after:
                            rope(dst, dst_lo, s_lo)
                    yield fin
                return ops()

            # ---- V transpose micro-ops for one s-group (PE + DVE copy) ----
            def vtrans_ops(sg):
                def ops():
                    for tj in range(4 * sg, 4 * sg + 4):
                        tp = ps_tile("oproj", 2, (128, 128), bf)

                        def one(tj=tj, tp=tp):
                            nc.tensor.transpose(
                                tp[:], vT_sb[:, tj * 128:(tj + 1) * 128],
                                ident[:])
                            nc.vector.tensor_copy(
                                vn_sb[:, tj * 128:(tj + 1) * 128], tp[:])
                        yield one
                return ops()

            # ---- o_proj micro-ops for one (st, eg) output tile ----
            def oproj_ops(st, eg):
                def ops():
                    t = ps_tile("oproj", 2)
                    for h in range(HPC):
                        yield lambda h=h, t=t: nc.tensor.matmul(
                            t[:],
                            at_sb[:, h * S + st * 128: h * S + st * 128 + 128],
                            wo_sb[:, h * HID + eg * 512: h * HID + eg * 512 + 512],
                            start=(h == 0), stop=(h == HPC - 1),
                        )

                    def fin(t=t):
                        ostage = ostage_pool.tile([128, 512], bf,
                                                  name="ostage")
                        nc.scalar.copy(ostage[:], t[:])
                        nc.sync.dma_start(
                            out[st * 128:(st + 1) * 128,
                                eg * 512:(eg + 1) * 512],
                            ostage[:])
                    yield fin
                return ops()

            # ---- filler machinery: a queue of PE micro-op generators ----
            # fills[i] = (key, generator); popping runs one micro-op.
            fills = []
            done_keys = set()

            def pop_fill(n=1):
                k = 0
                while fills and k < n:
                    key, gen = fills[0]
                    try:
                        next(gen)()
                        k += 1
                    except StopIteration:
                        done_keys.add(key)
                        fills.pop(0)
                return k

            def drain_until(key):
                while key not in done_keys:
                    if not pop_fill(1):
                        raise RuntimeError(f"filler {key} was never queued")

            def drain_all():
                while fills:
                    pop_fill(4)

            # Phase 1 tail: q head 0 / s-group 0, then v s-group 0 inline,
            # plus RoPE for the remaining k s-groups (DVE-only, overlaps).
            for op in proj_chain_ops(wq_sb, QDIM, 0, qT_sb, 0, 0, "qfill",
                                     True):
                op()
            for sg in range(1, 4):
                rope(kT_sb, sg * 512, sg * 512)
            for op in proj_chain_ops(wv_sb, HD, 0, vT_sb, 0, 0, "score",
                                     False):
                op()
            for op in vtrans_ops(0):
                op()

            # Filler queue: v s-groups 1..3 (+transposes), then q chains.
            for sg in range(1, 4):
                fills.append((f"v{sg}", proj_chain_ops(
                    wv_sb, HD, 0, vT_sb, sg * 512, sg * 512, "qfill",
                    False)))
                fills.append((f"vt{sg}", vtrans_ops(sg)))
            for h in range(1, HPC):
                fills.append((f"q{h}g0", proj_chain_ops(
                    wq_sb, QDIM, h, qT_sb, h * S, 0, "qfill", True)))
            for sg in range(1, 4):
                for h in range(HPC):
                    fills.append((f"q{h}g{sg}", proj_chain_ops(
                        wq_sb, QDIM, h, qT_sb, h * S + sg * 512, sg * 512,
                        "qfill", True)))
            done_keys.add("q0g0")
            done_keys.add("v0")
            done_keys.add("vt0")

            # ---- attention: gi-outer; o_proj(gi-1) drains as filler ----
            for gi in range(SB):
                for sg in range(1, gi + 1):
                    drain_until(f"v{sg}")
                    drain_until(f"vt{sg}")
                ntj = 4 * gi + 4
                for h in range(HPC):
                    drain_until(f"q{h}g{gi}")
                    qh = qT_sb[:, h * S:(h + 1) * S]
                    pv = ps_tile("pv", 2)
                    acc = acc_pool.tile([128, 512], f32r, name="acc")
                    for tj in range(ntj):
                        p = tj - 4 * gi  # >=0 on diagonal blocks
                        off = 128 * p if p > 0 else 0
                        w = 512 - off
                        sc = ps_tile("score", 2)
                        if p >= 0:
                            nc.vector.tensor_copy(
                                sc[:, off:512],
                                maskb_sb[:, p * 512 + off:(p + 1) * 512])
                        nc.tensor.matmul(
                            sc[:, off:512],
                            kT_sb[:, tj * 128:(tj + 1) * 128],
                            qh[:, gi * 512 + off: gi * 512 + 512],
                            start=(p < 0), stop=True,
                            skip_group_check=True,
                        )
                        pop_fill(1)
                        pb = probs_pool.tile([128, 512], bf, name="pb")
                        nc.scalar.activation(
                            pb[:, off:512], sc[:, off:512], Exp,
                            scale=inv_sqrt_d)
                        if tj == 0:
                            nc.vector.tensor_copy(acc[:], pb[:])
                        else:
                            nc.vector.scalar_tensor_tensor(
                                acc[:, off:512], pb[:, off:512], 1.0,
                                acc[:, off:512], MUL, ADD)
                        nc.tensor.matmul(
                            pv[:, off:512],
                            vn_sb[:, tj * 128:(tj + 1) * 128],
                            pb[:, off:512],
                            start=(tj == 0), stop=(tj == ntj - 1),
                            skip_group_check=True,
                        )
                        pop_fill(2 if gi >= 2 else 1)
                    # softmax denominators: fold partitions with one f32r
                    # matmul, reciprocal, Pool-broadcast back to 128 rows.
                    fold = ps_tile("small", 1, (1, 512))
                    nc.tensor.matmul(
                        fold[:], ones_f32[:], acc[:],
                        start=True, stop=True)
                    rc1 = rc_pool.tile([1, 512], f32r, name="rc1")
                    with nc.allow_low_precision(
                            reason="f32r reciprocal of softmax sums"):
                        nc.vector.reciprocal(rc1[:], fold[:])
                    bc = ps_tile("small", 1)
                    nc.tensor.matmul(
                        bc[:], onesr_sb[:], rc1[:], start=True, stop=True)
                    a_sl = at_sb[:, h * S + gi * 512: h * S + gi * 512 + 512]
                    nc.vector.tensor_copy(a_sl, pv[:])
                    nc.vector.tensor_mul(a_sl, a_sl, bc[:])
                    pop_fill(2)
                # o_proj for the 4 s-tiles of this i-group becomes filler
                # work for the next i-group (drained inline for the last).
                for st in range(4 * gi, 4 * gi + 4):
                    for eg in range(SB):
                        fills.append((f"o{st}e{eg}", oproj_ops(st, eg)))
            drain_all()

    _legalize_waits(nc)
    return nc


def _host_prep(hidden_states, Wq, Wk, Wv, Wo, position_ids):
    bf = ml_dtypes.bfloat16
    inv_freq = 1.0 / (THETA ** (np.arange(0, HD, 2, dtype=np.float64) / HD))

    # additive causal bias for the 4 diagonal-block phases: 0 keep, -1e9 mask
    maskb = np.zeros((4, 128, 512), dtype=np.float32)
    jl = np.arange(128)[:, None]
    il = np.arange(512)[None, :]
    for p in range(4):
        maskb[p] = np.where(128 * p + jl <= il, 0.0, -1e9)
    maskb = maskb.astype(bf)

    in_maps = []
    for d in range(NCORES):
        b, g = d // 4, d % 4
        hsT = np.ascontiguousarray(hidden_states[b].T).astype(bf).reshape(KT, 128, S)
        wqT = np.ascontiguousarray(Wq[g * QDIM:(g + 1) * QDIM].T).astype(bf).reshape(KT, 128, QDIM)
        wkT = np.ascontiguousarray(Wk[g * HD:(g + 1) * HD].T).astype(bf).reshape(KT, 128, HD)
        wvT = np.ascontiguousarray(Wv[g * HD:(g + 1) * HD].T).astype(bf).reshape(KT, 128, HD)
        woT = np.ascontiguousarray(Wo[:, g * QDIM:(g + 1) * QDIM].T).astype(bf).reshape(4, 128, HID)
        freqs = position_ids[b].astype(np.float64)[:, None] * inv_freq[None, :]  # [S, 64]
        emb = np.concatenate([freqs, freqs], axis=1)  # [S, 128]
        cosT = np.cos(emb).T.astype(bf)
        sinT = np.sin(emb).T.astype(bf)
        in_maps.append({
            "hsT": hsT, "wqT": wqT, "wkT": wkT, "wvT": wvT, "woT": woT,
            "cosT": np.ascontiguousarray(cosT),
            "sinT": np.ascontiguousarray(sinT),
            "maskb": maskb,
        })
    return in_maps


def kernel(hidden_states, Wq, Wk, Wv, Wo, position_ids, _trace=False, _tmpdir=None):
    from concourse.bass_utils import run_bass_kernel_spmd

    if "nc" not in _CACHE:
        _CACHE["nc"] = _build_nc()
    nc = _CACHE["nc"]

    in_maps = _host_prep(
        np.asarray(hidden_states), np.asarray(Wq), np.asarray(Wk),
        np.asarray(Wv), np.asarray(Wo), np.asarray(position_ids),
    )
    res = run_bass_kernel_spmd(
        nc, in_maps, core_ids=list(range(NCORES)), trace=_trace, tmpdir=_tmpdir
    )
    _CACHE["last_result"] = res

    out = np.zeros((B, S, NH * HD), dtype=np.float32)
    for d in range(NCORES):
        out[d // 4] += np.asarray(res.results[d]["out"], dtype=np.float32)
    return out
